# revision 14
# baseline (speedup 1.0000x reference)
"""DeepAR network (2-layer LSTM encoder/decoder + gaussian head) on 8 trn2 cores.

Strategy: pure data parallel. Batch 4096 -> 512 per core. Per core the two
LSTM layers are fused into one "tick" pipeline:
  tick t computes layer0 step t and layer1 step t-1 using one stacked state
  H = [h0;h1] (128 partitions x 512 batch, bf16) and C = [c0;c1] (fp32).
Gate pre-activations for both layers are produced by 2 accumulating matmuls
per gate tile (x-features K=23, recurrent K=128) into PSUM (128, 4*512).
Each tick is split into two batch halves (256 cols) so the two independent
recurrence chains interleave across engines (ACT is the throughput limit).

Host-side work is layout only: transposes to time-major, embedding row
gather, weight repacking, sharding, and final gather/transpose of outputs.
"""

import os
import sys

import numpy as np

sys.path.insert(0, "/opt/trn_rl_repo")

import ml_dtypes  # noqa: E402

BF = ml_dtypes.bfloat16

# ---------------------------------------------------------------- constants
B, C, H = 4096, 256, 24
HID, F, E = 64, 5, 10
NCORES = 8
BC = B // NCORES          # 512 batch per core
HB = BC // 2              # 256 half-batch
LAGS = [1, 2, 3, 7, 14]
PAD = 16                  # zero rows in front of the scaled-series table
TC = 16                   # ticks per x-feature chunk
LOG_2PI = 1.8378770664093453

# XF row layout (23 rows):
#   0..3  series lags [3,2,1,0]  (row3 = current target (enc) / prev (dec))
#   4     lag 7
#   5     lag 14
#   6..10 time features
#   11    log1p(age)
#   12..21 item embedding (constant over time)
#   22    ones (bias row)
KX = 23
# map XF row -> reference W_ih0 column
COLPERM = [3, 2, 1, 0, 4, 5, 6, 7, 8, 9, 10, 21]  # rows 0..11
# rows 12..21 -> W cols 11..20 (embedding), row 22 -> bias

# gate tile order in PSUM free dim: i, f, o, g  (torch row-blocks 0,1,3,2)
GATE_BLOCKS = [0, 1, 3, 2]

_BUILT = None  # cached (nc, names) after first build


# ================================================================ device IR
def build_module(c_steps=C, h_steps=H, debug=False):
    from concourse import bacc, tile, mybir

    F32 = mybir.dt.float32
    BF16 = mybir.dt.bfloat16
    Act = mybir.ActivationFunctionType
    Alu = mybir.AluOpType

    n_ticks = c_steps + h_steps + 1
    n_chunks = (n_ticks + TC - 1) // TC
    # scaled-series table rows: PAD zeros, c_steps past, h_steps future, pad tail
    st_rows = PAD + n_chunks * TC + TC
    age_rows = n_chunks * TC + TC

    nc = bacc.Bacc("TRN2", target_bir_lowering=False, debug=debug, num_devices=1)

    # ---- external inputs (per-core shards, host-prepared layouts)
    ptT = nc.dram_tensor("ptT", (c_steps, BC), F32, kind="ExternalInput").ap()
    ftT = nc.dram_tensor("ftT", (h_steps, BC), F32, kind="ExternalInput").ap()
    ageT = nc.dram_tensor("ageT", (age_rows, BC), F32, kind="ExternalInput").ap()
    tfT = nc.dram_tensor("tfT", (F, c_steps * BC), BF16, kind="ExternalInput").ap()
    ftfT = nc.dram_tensor("ftfT", (F, 2 * TC * BC), BF16, kind="ExternalInput").ap()
    embrep = nc.dram_tensor("embrep", (E + 1, TC * BC), BF16, kind="ExternalInput").ap()
    Wxs = nc.dram_tensor("Wxs", (KX, 512), BF16, kind="ExternalInput").ap()
    WHs = nc.dram_tensor("WHs", (128, 512), BF16, kind="ExternalInput").ap()
    Wms = nc.dram_tensor("Wms", (128, 2), BF16, kind="ExternalInput").ap()
    bms = nc.dram_tensor("bms", (h_steps, 2), F32, kind="ExternalInput").ap()

    # ---- external outputs
    muT_o = nc.dram_tensor("muT", (h_steps, BC), F32, kind="ExternalOutput").ap()
    sigT_o = nc.dram_tensor("sigT", (h_steps, BC), F32, kind="ExternalOutput").ap()
    scale_o = nc.dram_tensor("scale", (1, BC), F32, kind="ExternalOutput").ap()
    lsum_o = nc.dram_tensor("lsum", (1, 1), F32, kind="ExternalOutput").ap()

    # ---- internal DRAM staging (time-major scaled series / log-age)
    sT = nc.dram_tensor("sT", (st_rows, BC), BF16, kind="Internal").ap()
    ageL = nc.dram_tensor("ageL", (age_rows, BC), BF16, kind="Internal").ap()

    with tile.TileContext(nc) as tc:
        _emit(nc, tc, mybir, c_steps, h_steps, n_ticks, n_chunks,
              ptT, ftT, ageT, tfT, ftfT, embrep, Wxs, WHs, Wms, bms,
              muT_o, sigT_o, scale_o, lsum_o, sT, ageL)
    nc.compile()
    return nc


def _emit(nc, tc, mybir, c_steps, h_steps, n_ticks, n_chunks,
          ptT, ftT, ageT, tfT, ftfT, embrep, Wxs, WHs, Wms, bms,
          muT_o, sigT_o, scale_o, lsum_o, sT, ageL):
    from contextlib import ExitStack

    F32 = mybir.dt.float32
    BF16 = mybir.dt.bfloat16
    Act = mybir.ActivationFunctionType
    Alu = mybir.AluOpType

    ctx = ExitStack()
    with ctx:
        cp = ctx.enter_context(tc.tile_pool(name="const", bufs=1))

        # ---------------- load weights / small consts
        wx = cp.tile([KX, 512], BF16)
        nc.sync.dma_start(wx[:], Wxs[:])
        wh = cp.tile([128, 512], BF16)
        nc.sync.dma_start(wh[:], WHs[:])
        wms = cp.tile([128, 2], BF16)
        nc.sync.dma_start(wms[:], Wms[:])
        bmt = cp.tile([h_steps, 2], F32)
        nc.sync.dma_start(bmt[:], bms[:])

        # ---------------- scale = clip(mean |past|, 1e-5) ; recip broadcast
        n_pt = (c_steps + 127) // 128
        pt_tiles = []
        for i in range(n_pt):
            r = min(128, c_steps - 128 * i)
            t = cp.tile([r, BC], F32, tag=f"ptT{i}")
            nc.sync.dma_start(t[:], ptT[128 * i:128 * i + r, :])
            pt_tiles.append((t, r))
        ft_t = cp.tile([h_steps, BC], F32)
        nc.sync.dma_start(ft_t[:], ftT[:])

        ones = cp.tile([128, 1], F32)
        nc.gpsimd.memset(ones[:], 1.0)

        with tc.tile_pool(name="spsum", bufs=1, space="PSUM") as sp:
            ssum = sp.tile([1, BC], F32)
            abs_tiles = []
            for i, (t, r) in enumerate(pt_tiles):
                a = cp.tile([r, BC], F32, tag=f"abs{i}")
                nc.scalar.activation(a[:], t[:], Act.Abs)
                abs_tiles.append((a, r))
            for i, (a, r) in enumerate(abs_tiles):
                nc.tensor.matmul(ssum[:], ones[:r, :], a[:],
                                 start=(i == 0), stop=(i == len(abs_tiles) - 1))
            scl = cp.tile([1, BC], F32)
            nc.vector.tensor_scalar(scl[:], ssum[:], 1.0 / c_steps, 1e-5,
                                    Alu.mult, Alu.max)
        nc.sync.dma_start(scale_o[:], scl[:])
        rec = cp.tile([1, BC], F32)
        nc.vector.reciprocal(rec[:], scl[:])
        recB = cp.tile([128, BC], F32)
        nc.gpsimd.partition_broadcast(recB[:], rec[:])

        # ---------------- scaled series (bf16) -> sT staging in DRAM
        zt = cp.tile([PAD, BC], BF16)
        nc.gpsimd.memset(zt[:], 0.0)
        nc.sync.dma_start(sT[0:PAD, :], zt[:])
        tail0 = PAD + c_steps + h_steps
        for r0 in range(tail0, sT.shape[0], PAD):
            r = min(PAD, sT.shape[0] - r0)
            nc.sync.dma_start(sT[r0:r0 + r, :], zt[:r, :])
        for i, (t, r) in enumerate(pt_tiles):
            s = cp.tile([r, BC], BF16, tag=f"sc{i}")
            nc.vector.tensor_tensor(s[:], t[:], recB[:r, :], Alu.mult)
            nc.sync.dma_start(sT[PAD + 128 * i:PAD + 128 * i + r, :], s[:])
        sf = cp.tile([h_steps, BC], BF16)
        nc.vector.tensor_tensor(sf[:], ft_t[:], recB[:h_steps, :], Alu.mult)
        nc.sync.dma_start(sT[PAD + c_steps:PAD + c_steps + h_steps, :], sf[:])
        y32 = cp.tile([h_steps, BC], F32)  # scaled future target, for the loss
        nc.vector.tensor_tensor(y32[:], ft_t[:], recB[:h_steps, :], Alu.mult)

        # ---------------- log1p(age) -> ageL staging in DRAM
        for r0 in range(0, ageT.shape[0], 128):
            r = min(128, ageT.shape[0] - r0)
            a = cp.tile([r, BC], F32, tag=f"agi{r0}")
            nc.sync.dma_start(a[:], ageT[r0:r0 + r, :])
            al = cp.tile([r, BC], BF16, tag=f"ago{r0}")
            nc.scalar.activation(al[:], a[:], Act.Ln, bias=1.0)
            nc.sync.dma_start(ageL[r0:r0 + r, :], al[:])

        # ---------------- persistent recurrent state
        hp = ctx.enter_context(tc.tile_pool(name="hst", bufs=2))
        cpl = ctx.enter_context(tc.tile_pool(name="cst", bufs=2))
        xfp = ctx.enter_context(tc.tile_pool(name="xf", bufs=2))
        sgp = ctx.enter_context(tc.tile_pool(name="sg", bufs=3))
        tgp = ctx.enter_context(tc.tile_pool(name="tg", bufs=3))
        tcp = ctx.enter_context(tc.tile_pool(name="tc_", bufs=3))
        uvp = ctx.enter_context(tc.tile_pool(name="uv", bufs=3))
        t1p = ctx.enter_context(tc.tile_pool(name="t1", bufs=3))
        msp = ctx.enter_context(tc.tile_pool(name="ms", bufs=1))

        h_prev = hp.tile([128, BC], BF16, tag="h")
        nc.gpsimd.memset(h_prev[:], 0.0)
        c_prev = cpl.tile([128, BC], F32, tag="c")
        nc.gpsimd.memset(c_prev[:], 0.0)

        # flat (step-major) mu/sigma pre-activations written during decoder
        msf = msp.tile([2, h_steps * BC], F32, tag="msf")

        with tc.tile_pool(name="gps", bufs=3, space="PSUM") as gp, \
             tc.tile_pool(name="hdp", bufs=2, space="PSUM") as hd:

            xf_tile = None
            for tk in range(n_ticks):
                # ---- new x-feature chunk every TC ticks
                if tk % TC == 0:
                    k = tk // TC
                    t0 = tk  # global step of first tick in chunk
                    xf_tile = xfp.tile([KX, TC * BC], BF16, tag="xf")
                    dec = t0 >= c_steps
                    # series rows 0..5; each row is TC*BC contiguous elems of sT
                    lag_rows = ([3, 2, 1, 0] if not dec else [3, 2, 1, 1]) + [7, 14]
                    for r, lg in enumerate(lag_rows):
                        src0 = PAD + t0 - lg
                        nc.sync.dma_start(
                            xf_tile[r:r + 1, :],
                            sT[src0:src0 + TC, :])
                    # time features rows 6..10
                    if not dec:
                        nc.sync.dma_start(xf_tile[6:11, :],
                                          tfT[:, t0 * BC:(t0 + TC) * BC])
                    else:
                        o = (t0 - c_steps) * BC
                        nc.sync.dma_start(xf_tile[6:11, :],
                                          ftfT[:, o:o + TC * BC])
                    # age row 11
                    nc.sync.dma_start(xf_tile[11:12, :], ageL[t0:t0 + TC, :])
                    # embedding + ones rows 12..22
                    nc.sync.dma_start(xf_tile[12:KX, :], embrep[:])

                off = (tk % TC) * BC
                h_new = hp.tile([128, BC], BF16, tag="h")
                c_new = cpl.tile([128, BC], F32, tag="c")

                for hf in range(2):
                    hs0 = hf * HB
                    xv = xf_tile[:, off + hs0:off + hs0 + HB]
                    g = gp.tile([128, 4 * HB], F32, tag="g")
                    for gt in range(4):
                        gsl = g[:, gt * HB:(gt + 1) * HB]
                        nc.tensor.matmul(gsl, wx[:, gt * 128:(gt + 1) * 128],
                                         xv, start=True, stop=False)
                        nc.tensor.matmul(gsl, wh[:, gt * 128:(gt + 1) * 128],
                                         h_prev[:, hs0:hs0 + HB],
                                         start=False, stop=True)
                    sig = sgp.tile([128, 3 * HB], F32, tag="s")
                    nc.scalar.activation(sig[:], g[:, 0:3 * HB], Act.Sigmoid)
                    tg = tgp.tile([128, HB], F32, tag="t")
                    nc.scalar.activation(tg[:], g[:, 3 * HB:4 * HB], Act.Tanh)
                    u = uvp.tile([128, HB], F32, tag="u")
                    nc.vector.tensor_tensor(u[:], sig[:, 0:HB], tg[:], Alu.mult)
                    t1 = t1p.tile([128, HB], F32, tag="t1")
                    nc.gpsimd.tensor_tensor(t1[:], sig[:, HB:2 * HB],
                                            c_prev[:, hs0:hs0 + HB], Alu.mult)
                    cn = c_new[:, hs0:hs0 + HB]
                    nc.vector.tensor_tensor(cn, t1[:], u[:], Alu.add)
                    tc_ = tcp.tile([128, HB], F32, tag="tc")
                    nc.scalar.activation(tc_[:], cn, Act.Tanh)
                    nc.vector.tensor_tensor(h_new[:, hs0:hs0 + HB],
                                            sig[:, 2 * HB:3 * HB], tc_[:],
                                            Alu.mult)
                    # gaussian head on h1 of decoder steps
                    d = tk - (c_steps + 1)
                    if 0 <= d < h_steps:
                        ms = hd.tile([2, HB], F32, tag="ms")
                        nc.tensor.matmul(ms[:], wms[:],
                                         h_new[:, hs0:hs0 + HB],
                                         start=True, stop=True)
                        nc.scalar.copy(msf[:, d * BC + hs0:d * BC + hs0 + HB],
                                       ms[:])

                if tk == 0:
                    # layer1 "step -1" is garbage; reset its state to zero
                    nc.gpsimd.memset(h_new[64:128, :], 0.0)
                    nc.gpsimd.memset(c_new[64:128, :], 0.0)
                h_prev, c_prev = h_new, c_new

        # ---------------- head epilogue + NLL partial sum
        with tc.tile_pool(name="fpsum", bufs=1, space="PSUM") as fp:
            # reshape flat (2, h*BC) -> partition-major (h, BC) via sbuf DMA
            mupm = cp.tile([h_steps, BC], F32, tag="mupm")
            nc.sync.dma_start(mupm[:], msf[0:1, :])
            sgpm = cp.tile([h_steps, BC], F32, tag="sgpm")
            nc.sync.dma_start(sgpm[:], msf[1:2, :])
            msmu = cp.tile([h_steps, BC], F32, tag="msmu")
            nc.scalar.activation(msmu[:], mupm[:], Act.Identity,
                                 bias=bmt[:, 0:1])
            # softplus(x+b) = ln(exp(x+b) + 1); Exp/Ln/Square share a table set
            ex = cp.tile([h_steps, BC], F32, tag="ex")
            nc.scalar.activation(ex[:], sgpm[:], Act.Exp, bias=bmt[:, 1:2])
            sgf = cp.tile([h_steps, BC], F32)
            nc.scalar.activation(sgf[:], ex[:], Act.Ln, bias=1.0)
            nc.sync.dma_start(muT_o[:], msmu[:])
            nc.sync.dma_start(sigT_o[:], sgf[:])

            lnacc = cp.tile([h_steps, 1], F32)
            lnout = cp.tile([h_steps, BC], F32, tag="lnout")
            nc.scalar.activation(lnout[:], sgf[:], Act.Ln, accum_out=lnacc[:])
            rs = cp.tile([h_steps, BC], F32, tag="rs")
            nc.vector.reciprocal(rs[:], sgf[:])
            df = cp.tile([h_steps, BC], F32, tag="df")
            nc.vector.tensor_tensor(df[:], y32[:], msmu[:], Alu.subtract)
            dd = cp.tile([h_steps, BC], F32, tag="dd")
            nc.vector.tensor_tensor(dd[:], df[:], rs[:], Alu.mult)
            sqacc = cp.tile([h_steps, 1], F32)
            sqout = cp.tile([h_steps, BC], F32, tag="sqout")
            nc.scalar.activation(sqout[:], dd[:], Act.Square, accum_out=sqacc[:])
            tot = cp.tile([h_steps, 1], F32)
            nc.vector.scalar_tensor_tensor(tot[:], sqacc[:], 0.5, lnacc[:],
                                           Alu.mult, Alu.add)
            ones24 = cp.tile([h_steps, 1], F32)
            nc.gpsimd.memset(ones24[:], 1.0)
            ls = fp.tile([1, 1], F32)
            nc.tensor.matmul(ls[:], ones24[:], tot[:], start=True, stop=True)
            lss = cp.tile([1, 1], F32)
            nc.scalar.copy(lss[:], ls[:])
            nc.sync.dma_start(lsum_o[:], lss[:])


# ============================================================== host prep
def _prep_core_inputs(inputs, core, c_steps=C, h_steps=H):
    """Build the per-core input map (pure layout: slice/transpose/cast)."""
    sl = slice(core * BC, (core + 1) * BC)
    n_ticks = c_steps + h_steps + 1
    n_chunks = (n_ticks + TC - 1) // TC
    age_rows = n_chunks * TC + TC

    pt = np.asarray(inputs["past_target"], np.float32)[sl]        # (BC, C)
    ft = np.asarray(inputs["future_target"], np.float32)[sl]      # (BC, H)
    pa = np.asarray(inputs["past_age"], np.float32)[sl]
    fa = np.asarray(inputs["future_age"], np.float32)[sl]
    ptf = np.asarray(inputs["past_time_features"], np.float32)[sl]   # (BC,C,F)
    ftf = np.asarray(inputs["future_time_features"], np.float32)[sl]  # (BC,H,F)
    idx = np.asarray(inputs["item_id_index"]).astype(np.int64)[sl]
    emb = np.asarray(inputs["emb_table"], np.float32)[idx]        # (BC, E)

    ageT = np.zeros((age_rows, BC), np.float32)
    ageT[:c_steps] = pa.T
    ageT[c_steps:c_steps + h_steps] = fa.T

    tfT = np.ascontiguousarray(ptf.transpose(2, 1, 0)).reshape(F, c_steps * BC)
    ftf_p = np.zeros((F, 2 * TC, BC), np.float32)
    ftf_p[:, :h_steps] = ftf.transpose(2, 1, 0)
    ftfT = ftf_p.reshape(F, 2 * TC * BC)

    er = np.ones((E + 1, TC, BC), np.float32)
    er[:E] = emb.T[:, None, :]
    embrep = er.reshape(E + 1, TC * BC)

    # ---- weights
    W_ih0 = np.asarray(inputs["W_ih0"], np.float32)
    W_hh0 = np.asarray(inputs["W_hh0"], np.float32)
    W_ih1 = np.asarray(inputs["W_ih1"], np.float32)
    W_hh1 = np.asarray(inputs["W_hh1"], np.float32)
    b0 = np.asarray(inputs["b_ih0"], np.float32) + np.asarray(inputs["b_hh0"], np.float32)
    b1 = np.asarray(inputs["b_ih1"], np.float32) + np.asarray(inputs["b_hh1"], np.float32)

    Wxs = np.zeros((KX, 512), np.float32)
    WHs = np.zeros((128, 512), np.float32)
    for gt, tg in enumerate(GATE_BLOCKS):
        r0 = 64 * tg
        m0 = 128 * gt
        # x-part: layer0 half gets W_ih0 (permuted cols); layer1 only bias
        Wxs[:12, m0:m0 + 64] = W_ih0[r0:r0 + 64, COLPERM].T
        Wxs[12:22, m0:m0 + 64] = W_ih0[r0:r0 + 64, 11:21].T
        Wxs[22, m0:m0 + 64] = b0[r0:r0 + 64]
        Wxs[22, m0 + 64:m0 + 128] = b1[r0:r0 + 64]
        # H-part: K rows 0..63 = h0, 64..127 = h1
        WHs[0:64, m0:m0 + 64] = W_hh0[r0:r0 + 64, :].T
        WHs[0:64, m0 + 64:m0 + 128] = W_ih1[r0:r0 + 64, :].T
        WHs[64:128, m0 + 64:m0 + 128] = W_hh1[r0:r0 + 64, :].T

    Wms = np.zeros((128, 2), np.float32)
    Wms[64:, 0] = np.asarray(inputs["W_mu"], np.float32)[0]
    Wms[64:, 1] = np.asarray(inputs["W_sigma"], np.float32)[0]
    bms = np.zeros((h_steps, 2), np.float32)
    bms[:, 0] = float(np.asarray(inputs["b_mu"]).reshape(-1)[0])
    bms[:, 1] = float(np.asarray(inputs["b_sigma"]).reshape(-1)[0])

    return {
        "ptT": np.ascontiguousarray(pt.T),
        "ftT": np.ascontiguousarray(ft.T),
        "ageT": ageT,
        "tfT": tfT.astype(BF),
        "ftfT": ftfT.astype(BF),
        "embrep": embrep.astype(BF),
        "Wxs": Wxs.astype(BF),
        "WHs": WHs.astype(BF),
        "Wms": Wms.astype(BF),
        "bms": bms,
    }


def _get_module():
    global _BUILT
    if _BUILT is None:
        _BUILT = build_module()
    return _BUILT


def kernel(**inputs):
    from concourse import bass_utils

    nc = _get_module()
    in_maps = [_prep_core_inputs(inputs, c) for c in range(NCORES)]
    res = bass_utils.run_bass_kernel_spmd(nc, in_maps, core_ids=list(range(NCORES)))
    outs = res.results

    mu = np.concatenate([o["muT"].T for o in outs], axis=0).astype(np.float32)
    sig = np.concatenate([o["sigT"].T for o in outs], axis=0).astype(np.float32)
    scale = np.concatenate([o["scale"].T for o in outs], axis=0).astype(np.float32)
    tot = float(sum(float(o["lsum"][0, 0]) for o in outs))
    loss = np.float32(0.5 * LOG_2PI + tot / (B * H))
    return loss, mu, sig, scale


def timed_run(inputs):
    """Test-only: run once with NTFF tracing and return HW exec time in ns."""
    from concourse import bass_utils

    nc = _get_module()
    in_maps = [_prep_core_inputs(inputs, c) for c in range(NCORES)]
    res = bass_utils.run_bass_kernel_spmd(
        nc, in_maps, core_ids=list(range(NCORES)), trace=True)
    return res.exec_time_ns
